# revision 3
# baseline (speedup 1.0000x reference)
"""Trainium2 Bass kernel for nn_CAttention (channel attention).

Reference computation (per batch b):
    k      = einsum('cit,i->ct', x[b], alpha)          # [C, T]
    scores = k @ W @ k.T                               # [C, C]
    att    = softmax(scores, axis=-1)
    out[b] = att @ x[b].reshape(C, N*T)                # [C, N*T]

Shapes (hardcoded): x [64, 256, 307, 12] f32, W [12, 12], alpha [307].
Sharding: data-parallel over batch B across 8 cores (8 batches/core);
W and alpha replicated.

The kernel is HBM-DMA bound: 30.2 MB in + 30.2 MB out per core across
16 DMA engines capped at ~24 GB/s each gives a ~158 us floor.  The
design keeps those engines saturated end to end:
 - x loads are issued from the SP (sync) sequencer, output stores from
   the Activation sequencer — separate hardware-DGE rings, so a load
   issue that blocks on x-buffer reuse can never head-of-line block a
   ready store.
 - Output rows are staged as full [128, 3684] SBUF rows and stored with
   one DMA per c-chunk: 14.7 KB per-partition descriptors.
 - Each x c-chunk load is split in two at the node boundary i=154 so
   the k-pooling starts while the rest of the batch still streams.
 - k = sum_i alpha_i * x[:, i, :] runs fully unit-stride: the alpha
   multiply writes an [128, 307, 12] scratch in natural layout (Pool
   engine takes i<154, DVE takes the rest), then each engine folds its
   half with an in-place halving tree (add of two contiguous blocks),
   so no strided scratch writes anywhere.
 - The big matmul runs in float32r (fp32 truncated to 11 mantissa bits
   at the PE) which streams 1 cycle/column like bf16 for >=256-wide
   moving tiles; x is DMA'd into a float32r-typed tile so the k-path
   reads the same bits as full fp32.
 - Softmax needs no transpose: scoresT [d, c] is computed directly,
   exp() writes attT in place as the big-matmul stationary, and the
   denominator comes from a ones-column appended to x — the big matmul
   emits sum_d exp(scores[c,d]) as an extra output column, and the
   normalization folds into the PSUM->SBUF copy.  exp() skips
   max-subtraction: |scores| <= ~30 here, far below fp32 overflow.
 - PSUM is used as 2 waves of 4 banks per c-chunk so the PE never
   waits on PSUM->SBUF copies; the dc accumulation order zigzags
   between waves so walrus (ldw-opt) elides the LDWEIGHTS at each wave
   boundary that shares its stationary operand.
"""

from contextlib import ExitStack

import numpy as np

import concourse.bass as bass
import concourse.bass_utils as _bass_utils
import concourse.tile as tile
from concourse import bacc, mybir
from concourse.bass import ts
from concourse.bass_utils import run_bass_kernel_spmd
from concourse.masks import make_identity

B, C, N, T = 64, 256, 307, 12
NCORES = 8
B_LOC = B // NCORES          # 8 batches per core
F = N * T                    # 3684 flattened free dim
FW = F + 2                   # + ones col (denominator) + pad col
P = 128                      # partitions
CC = C // P                  # 2 c-chunks
NS = 154                     # node split: Pool engine takes i<154, DVE the rest
FA = NS * T                  # 1848 columns in the first load chunk

# f-tiles of the big matmul: the tile holding the appended ones-column
# (softmax denominator, output col 3684 -> offset 450) goes first so the
# normalizer is ready before any PSUM->SBUF copy.  All widths even and
# >=256 so float32r streams at 1 cycle/column.
_FTILES = [(3234, 452)] + [(i * 462, 462) for i in range(7)]
_WAVES = [_FTILES[:4], _FTILES[4:]]
_DEN_OFF = F - 3234          # 450: denominator offset inside tile 0

_DT = mybir.dt.float32
_R = mybir.dt.float32r


def _enable_ldw_opt():
    """Compile with --enable-ldw-opt=true so walrus elides LDWEIGHTS for
    consecutive matmuls sharing the stationary operand.  bass_utils
    hardcodes false; float32r cannot use standalone ldweights, so this
    is the only way to amortize 4-byte weight loads."""
    if getattr(_bass_utils, "_ldw_opt_patched", False):
        return
    orig = _bass_utils.bir_verify_and_optimise

    def patched(tmpdir, inp="bir.json", outp="file.neff", arch=None, *, dve_root=None):
        real_run = _bass_utils.run_command

        def run_hook(argv, **kw):
            argv = [
                "--enable-ldw-opt=true" if a == "--enable-ldw-opt=false" else a
                for a in argv
            ]
            return real_run(argv, **kw)

        _bass_utils.run_command = run_hook
        try:
            return orig(tmpdir, inp, outp, arch, dve_root=dve_root)
        finally:
            _bass_utils.run_command = real_run

    _bass_utils.bir_verify_and_optimise = patched
    _bass_utils._ldw_opt_patched = True


def _emit_core_kernel(tc, x_ap, w_ap, alpha_ap, out_ap):
    """Emit the per-core program. x_ap/out_ap: [B_LOC, C, N, T] DRAM."""
    nc = tc.nc
    ctx = ExitStack()

    x_flat = x_ap.rearrange("b c i t -> b c (i t)")      # [B_LOC, C, F]
    out_flat = out_ap.rearrange("b c i t -> b c (i t)")  # [B_LOC, C, F]

    consts = ctx.enter_context(tc.tile_pool(name="consts", bufs=1))
    xpool = ctx.enter_context(tc.tile_pool(name="x", bufs=4))
    xapool = ctx.enter_context(tc.tile_pool(name="xa", bufs=2))
    kpool = ctx.enter_context(tc.tile_pool(name="k", bufs=3))
    ktpool = ctx.enter_context(tc.tile_pool(name="kt", bufs=4))
    attpool = ctx.enter_context(tc.tile_pool(name="att", bufs=3))
    outpool = ctx.enter_context(tc.tile_pool(name="out", bufs=2))
    rpool = ctx.enter_context(tc.tile_pool(name="rinv", bufs=4))
    # one full PSUM bank per tile, 8 banks total
    psum = ctx.enter_context(tc.tile_pool(name="psum", bufs=8, space="PSUM"))

    # Constants: identity for PE transpose, alpha broadcast, W.
    ident = consts.tile([P, P], _DT)
    make_identity(nc, ident)
    alpha_row = consts.tile([P, N], _DT)
    nc.gpsimd.dma_start(out=alpha_row, in_=alpha_ap[None, :].to_broadcast([P, N]))
    w_sb = consts.tile([T, T], _DT)
    nc.gpsimd.dma_start(out=w_sb, in_=w_ap)

    def tree_fold(eng, xa, base, n):
        """Fold xa[:, base:base+n, :] into xa[:, base, :] by repeated
        in-place adds of two contiguous blocks (all unit-stride)."""
        while n > 1:
            h = n // 2
            eng.tensor_add(
                xa[:, base : base + h, :],
                xa[:, base : base + h, :],
                xa[:, base + n - h : base + n, :],
            )
            n -= h

    def phase1a(b):
        """Load x[b] (split loads), compute k via Pool/DVE split trees."""
        x_t = xpool.tile([P, CC, FW], _R, tag="x")
        for cc in range(CC):
            nc.sync.dma_start(
                out=x_t[:, cc, :FA], in_=x_flat[b, ts(cc, P), :FA].bitcast(_R)
            )
            nc.sync.dma_start(
                out=x_t[:, cc, FA:F], in_=x_flat[b, ts(cc, P), FA:].bitcast(_R)
            )
            # ones-columns: big-matmul output col F = softmax denominator;
            # col F+1 pads the float32r moving dim to an even width.
            nc.gpsimd.memset(x_t[:, cc, F:FW].bitcast(_DT), 1.0)

        k_c = kpool.tile([P, CC, T], _DT, tag="k")
        for cc in range(CC):
            xa = xapool.tile([P, N, T], _DT, tag="xa")
            x_cc = x_t[:, cc, :F].bitcast(_DT).rearrange("p (i t) -> p i t", t=T)
            nc.gpsimd.tensor_mul(
                xa[:, :NS, :],
                x_cc[:, :NS, :],
                alpha_row[:, :NS, None].to_broadcast([P, NS, T]),
            )
            nc.vector.tensor_mul(
                xa[:, NS:, :],
                x_cc[:, NS:, :],
                alpha_row[:, NS:, None].to_broadcast([P, N - NS, T]),
            )
            tree_fold(nc.gpsimd, xa, 0, NS)
            tree_fold(nc.vector, xa, NS, N - NS)
            nc.vector.tensor_add(k_c[:, cc, :], xa[:, 0, :], xa[:, NS, :])
        return {"x_t": x_t, "k_c": k_c}

    def phase1b(b, st):
        """kT, kWT, scoresT, attT = exp(scoresT) — short PE/ACT chain."""
        k_c = st["k_c"]
        ps_kt = psum.tile([P, 512], _DT, tag="ps")
        nc.tensor.transpose(ps_kt[:T, 0:P], k_c[:, 0, :], ident)
        nc.tensor.transpose(ps_kt[:T, P:C], k_c[:, 1, :], ident)
        kt_sb = ktpool.tile([T, C], _DT, tag="kt")
        nc.scalar.copy(out=kt_sb, in_=ps_kt[:T, :C])

        # kWT[s, c] = sum_t W[t, s] kT[t, c]
        ps_kwt = psum.tile([P, 512], _DT, tag="ps")
        nc.tensor.matmul(ps_kwt[:T, :C], lhsT=w_sb, rhs=kt_sb, start=True, stop=True)
        kwt_sb = ktpool.tile([T, C], _DT, tag="kwt")
        nc.scalar.copy(out=kwt_sb, in_=ps_kwt[:T, :C])

        # scoresT[d, c] = sum_s kT[s, d] kWT[s, c]  (= scores[c, d]);
        # attT = exp(scoresT), written directly as float32r matmul weights.
        ps_sc = psum.tile([P, 512], _DT, tag="ps")
        att_t = attpool.tile([P, CC, C], _R, tag="attT")
        for dc in range(CC):
            nc.tensor.matmul(
                ps_sc[:, ts(dc, C)], lhsT=kt_sb[:, ts(dc, P)], rhs=kwt_sb,
                start=True, stop=True,
            )
        for dc in range(CC):
            nc.scalar.activation(
                out=att_t[:, dc, :],
                in_=ps_sc[:, ts(dc, C)],
                func=mybir.ActivationFunctionType.Exp,
            )
        st["att_t"] = att_t

    def phase2(b, st):
        """Big matmul out[c, f] (+ denominator column), normalize, store."""
        x_t, att_t = st["x_t"], st["att_t"]
        for cc in range(CC):
            rinv = rpool.tile([P, 1], _DT, tag="rinv")
            o_row = outpool.tile([P, F], _DT, tag="o")
            for wi, wave in enumerate(_WAVES):
                tiles = [
                    psum.tile([P, 512], _DT, tag="ps", name=f"ps_w{wi}_{i}")
                    for i in range(len(wave))
                ]
                # zigzag dc order: adjacent waves share the stationary at
                # the boundary, so ldw-opt drops that LDWEIGHTS.
                dcs = (0, 1) if wi == 0 else (1, 0)
                for j, dc in enumerate(dcs):
                    for (f0, fsz), pt in zip(wave, tiles):
                        nc.tensor.matmul(
                            pt[:, :fsz],
                            lhsT=att_t[:, dc, ts(cc, P)],
                            rhs=x_t[:, dc, f0 : f0 + fsz],
                            start=(j == 0),
                            stop=(j == 1),
                        )
                if wi == 0:
                    # col 3684 (offset 450 of tile 0) = sum_d exp(scores)
                    nc.vector.reciprocal(
                        out=rinv, in_=tiles[0][:, _DEN_OFF : _DEN_OFF + 1]
                    )
                for (f0, fsz), pt in zip(wave, tiles):
                    osz = min(fsz, F - f0)  # drop the ones-columns
                    nc.scalar.mul(
                        out=o_row[:, f0 : f0 + osz], in_=pt[:, :osz], mul=rinv
                    )
            # one full-row store per c-chunk, issued from the ACT ring so
            # it can never queue behind a blocked x-load issue.
            nc.scalar.dma_start(out=out_flat[b, ts(cc, P), :], in_=o_row)

    # Two-stage-lag software pipeline.  Per step: the big matmul of b-2
    # first (its inputs are long ready, keeps PE/ACT/DMA busy), then the
    # short scores chain of b-1, then loads + k of b.
    states = {}
    for s in range(B_LOC + 2):
        if 0 <= s - 2:
            phase2(s - 2, states.pop(s - 2))
        if 0 <= s - 1 < B_LOC:
            phase1b(s - 1, states[s - 1])
        if s < B_LOC:
            states[s] = phase1a(s)
    ctx.close()


_CACHED_NC = None


def _build():
    global _CACHED_NC
    if _CACHED_NC is not None:
        return _CACHED_NC
    _enable_ldw_opt()
    nc = bacc.Bacc("TRN2", target_bir_lowering=False, debug=False, num_devices=NCORES)
    x_d = nc.dram_tensor("x", [B_LOC, C, N, T], _DT, kind="ExternalInput").ap()
    w_d = nc.dram_tensor("W", [T, T], _DT, kind="ExternalInput").ap()
    a_d = nc.dram_tensor("alpha", [N], _DT, kind="ExternalInput").ap()
    o_d = nc.dram_tensor("out", [B_LOC, C, N, T], _DT, kind="ExternalOutput").ap()
    with tile.TileContext(nc) as tc:
        _emit_core_kernel(tc, x_d, w_d, a_d, o_d)
    nc.compile()
    _CACHED_NC = nc
    return nc


def run(x, W, alpha, trace=False, **spmd_kwargs):
    """Run on 8 cores; returns (full output [B,C,N,T], BassKernelResults)."""
    x = np.ascontiguousarray(np.asarray(x, dtype=np.float32))
    W = np.ascontiguousarray(np.asarray(W, dtype=np.float32))
    alpha = np.ascontiguousarray(np.asarray(alpha, dtype=np.float32))
    assert x.shape == (B, C, N, T) and W.shape == (T, T) and alpha.shape == (N,)

    nc = _build()
    in_maps = [
        {"x": x[i * B_LOC : (i + 1) * B_LOC], "W": W, "alpha": alpha}
        for i in range(NCORES)
    ]
    res = run_bass_kernel_spmd(
        nc, in_maps, core_ids=list(range(NCORES)), trace=trace, **spmd_kwargs
    )
    out = np.concatenate([r["out"] for r in res.results], axis=0)
    return out, res


def kernel(x, W, alpha):
    out, _ = run(x, W, alpha)
    return out


# revision 21
# speedup vs baseline: 1.3517x; 1.3517x over previous
"""Trainium2 Bass kernel for nn_CAttention (channel attention).

Reference computation (per batch b):
    k      = einsum('cit,i->ct', x[b], alpha)          # [C, T]
    scores = k @ W @ k.T                               # [C, C]
    att    = softmax(scores, axis=-1)
    out[b] = att @ x[b].reshape(C, N*T)                # [C, N*T]

Shapes (hardcoded): x [64, 256, 307, 12] f32, W [12, 12], alpha [307].
Sharding: data-parallel over batch B across 8 cores (8 batches/core);
W and alpha replicated.

The kernel is HBM-DMA bound: 30.2 MB in + 30.2 MB out per core across
16 DMA engines capped at ~24 GB/s each gives a ~158 us floor.  The
design keeps those engines saturated end to end:
 - x loads are issued from the SP (sync) sequencer, output stores from
   the Activation sequencer — separate hardware-DGE rings, so a load
   issue that blocks on x-buffer reuse can never head-of-line block a
   ready store.
 - Output rows are staged as full [128, 3684] SBUF rows and stored with
   one DMA per c-chunk: 14.7 KB per-partition descriptors.
 - Each x c-chunk load is split in two at the node boundary i=154 so
   the k-pooling starts while the rest of the batch still streams.
 - k = sum_i alpha_i * x[:, i, :]: the alpha multiply writes t-major
   scratch split across Pool (i<188) and DVE (the rest), then DVE
   reduces both scratches with unit-stride reduce_sums (free-axis
   reduces are DVE-only; they measure ~3x cheaper per element than
   tensor_tensor, and Pool's bigger mul share rebalances the load).
 - The big matmul runs in float32r (fp32 truncated to 11 mantissa bits
   at the PE) which streams 1 cycle/column like bf16 for >=256-wide
   moving tiles; x is DMA'd into a float32r-typed tile so the k-path
   reads the same bits as full fp32.
 - Softmax needs no transpose: scoresT [d, c] is computed directly,
   exp() writes attT in place as the big-matmul stationary, and the
   denominator comes from a ones-column appended to x (written by a
   tiny ACT copy, so no k-path engine ever waits on x-buffer reuse) —
   the big matmul emits sum_d exp(scores[c,d]) as an extra output
   column and the normalization folds into the PSUM->SBUF copy.
   exp() skips max-subtraction: |scores| <= ~30 here.
 - PSUM: 6 banks cycle through big-matmul waves of 3/3/2 tiles, 2
   banks are dedicated to the scores chain, so the PE never interlocks
   with the ACT copies; the dc accumulation order zigzags between
   waves so walrus (ldw-opt) elides the LDWEIGHTS at each wave
   boundary that shares its stationary operand.
"""

from contextlib import ExitStack

import numpy as np

import concourse.bass as bass
import concourse.bass_utils as _bass_utils
import concourse.tile as tile
from concourse import bacc, mybir
from concourse.bass import ts
from concourse.bass_utils import run_bass_kernel_spmd
from concourse.masks import make_identity

B, C, N, T = 64, 256, 307, 12
NCORES = 8
B_LOC = B // NCORES          # 8 batches per core
F = N * T                    # 3684 flattened free dim
FW = F + 2                   # + ones col (softmax denominator) + pad col
P = 128                      # partitions
CC = C // P                  # 2 c-chunks
NS = 188                     # node split: Pool engine muls i<188, DVE the rest
FA = NS * T                  # 2256 columns in the first load chunk

# f-tiles of the big matmul, in PSUM waves of 3/3/2 banks (the other 2
# banks belong to the scores chain).  The tile covering the appended
# ones-column (denominator at output col 3684 -> offset 450) goes first
# so the normalizer is ready before any PSUM->SBUF copy.  All widths
# even and >=256 so float32r streams at 1 cycle/column.
_FTILES = [(3234, 452)] + [(i * 462, 462) for i in range(7)]
_WAVES = [_FTILES[:3], _FTILES[3:6], _FTILES[6:]]
_DEN_OFF = 450               # denominator column offset inside tile 0

_DT = mybir.dt.float32
_R = mybir.dt.float32r


def _enable_ldw_opt():
    """Compile with --enable-ldw-opt=true so walrus elides LDWEIGHTS for
    consecutive matmuls sharing the stationary operand.  bass_utils
    hardcodes false; float32r cannot use standalone ldweights, so this
    is the only way to amortize 4-byte weight loads."""
    if getattr(_bass_utils, "_ldw_opt_patched", False):
        return
    orig = _bass_utils.bir_verify_and_optimise

    def patched(tmpdir, inp="bir.json", outp="file.neff", arch=None, *, dve_root=None):
        real_run = _bass_utils.run_command

        def run_hook(argv, **kw):
            argv = [
                "--enable-ldw-opt=true" if a == "--enable-ldw-opt=false" else a
                for a in argv
            ]
            return real_run(argv, **kw)

        _bass_utils.run_command = run_hook
        try:
            return orig(tmpdir, inp, outp, arch, dve_root=dve_root)
        finally:
            _bass_utils.run_command = real_run

    _bass_utils.bir_verify_and_optimise = patched
    _bass_utils._ldw_opt_patched = True


def _emit_core_kernel(tc, x_ap, w_ap, alpha_ap, out_ap):
    """Emit the per-core program. x_ap/out_ap: [B_LOC, C, N, T] DRAM."""
    nc = tc.nc
    ctx = ExitStack()

    x_flat = x_ap.rearrange("b c i t -> b c (i t)")      # [B_LOC, C, F]
    out_flat = out_ap.rearrange("b c i t -> b c (i t)")  # [B_LOC, C, F]

    consts = ctx.enter_context(tc.tile_pool(name="consts", bufs=1))
    xpool = ctx.enter_context(tc.tile_pool(name="x", bufs=4))
    xgpool = ctx.enter_context(tc.tile_pool(name="xg", bufs=2))
    xvpool = ctx.enter_context(tc.tile_pool(name="xv", bufs=2))
    kpool = ctx.enter_context(tc.tile_pool(name="k", bufs=6))
    ktpool = ctx.enter_context(tc.tile_pool(name="kt", bufs=4))
    attpool = ctx.enter_context(tc.tile_pool(name="att", bufs=3))
    outpool = ctx.enter_context(tc.tile_pool(name="out", bufs=2))
    rpool = ctx.enter_context(tc.tile_pool(name="rinv", bufs=4))
    # PSUM: 6 banks cycle through the big-matmul waves, 2 banks are
    # dedicated to the scores chain so the two phases never interlock.
    psum = ctx.enter_context(tc.tile_pool(name="psum", bufs=6, space="PSUM"))
    psum_sc = ctx.enter_context(tc.tile_pool(name="psum_sc", bufs=2, space="PSUM"))

    # Constants: identity for PE transpose, alpha broadcast, W, ones.
    ident = consts.tile([P, P], _DT)
    make_identity(nc, ident)
    alpha_row = consts.tile([P, N], _DT)
    nc.gpsimd.dma_start(out=alpha_row, in_=alpha_ap[None, :].to_broadcast([P, N]))
    w_sb = consts.tile([T, T], _DT)
    nc.gpsimd.dma_start(out=w_sb, in_=w_ap)
    # ones moving operand for the softmax-denominator matmul columns
    ones_c = consts.tile([P, 2], _DT)
    nc.gpsimd.memset(ones_c, 1.0)

    def phase1a(b):
        """Load x[b] (split loads); k via Pool/DVE t-major mul + reduce.

        The alpha multiply writes t-major scratch (strided writes cost
        the same as natural-layout ones on these engines) so the
        i-reduction is a single unit-stride reduce_sum per engine —
        reduces measure ~3x cheaper per element than tensor_tensor."""
        x_t = xpool.tile([P, CC, FW], _R, tag="x")
        for cc in range(CC):
            nc.sync.dma_start(
                out=x_t[:, cc, :FA], in_=x_flat[b, ts(cc, P), :FA].bitcast(_R)
            )
            nc.sync.dma_start(
                out=x_t[:, cc, FA:F], in_=x_flat[b, ts(cc, P), FA:].bitcast(_R)
            )
            # ones-columns for the denominator; ACT copy (with fp32->
            # fp32r rounding on write, exact for 1.0) so no engine on
            # the k critical path ever waits on x-buffer reuse.
            nc.scalar.copy(out=x_t[:, cc, F:FW], in_=ones_c)

        k_c = kpool.tile([P, CC, T], _DT, tag="k")
        for cc in range(CC):
            xg = xgpool.tile([P, T, NS], _DT, tag="xg")
            xv = xvpool.tile([P, T, N - NS], _DT, tag="xv")
            ka = kpool.tile([P, 2, T], _DT, tag="ka")
            x_cc = x_t[:, cc, :F].bitcast(_DT).rearrange("p (i t) -> p i t", t=T)
            nc.gpsimd.tensor_mul(
                xg.rearrange("p t i -> p i t"),
                x_cc[:, :NS, :],
                alpha_row[:, :NS, None].to_broadcast([P, NS, T]),
            )
            nc.vector.tensor_mul(
                xv.rearrange("p t i -> p i t"),
                x_cc[:, NS:, :],
                alpha_row[:, NS:, None].to_broadcast([P, N - NS, T]),
            )
            # free-axis reduces are DVE-only (Pool can only reduce over C)
            nc.vector.reduce_sum(out=ka[:, 0, :], in_=xg, axis=mybir.AxisListType.X)
            nc.vector.reduce_sum(out=ka[:, 1, :], in_=xv, axis=mybir.AxisListType.X)
            nc.vector.tensor_add(k_c[:, cc, :], ka[:, 0, :], ka[:, 1, :])
        return {"x_t": x_t, "k_c": k_c}

    def phase1b(b, st):
        """kT, kWT, scoresT, attT = exp(scoresT) — short PE/ACT chain."""
        k_c = st["k_c"]
        ps_kt = psum_sc.tile([P, 512], _DT, tag="ps_sc")
        nc.tensor.transpose(ps_kt[:T, 0:P], k_c[:, 0, :], ident)
        nc.tensor.transpose(ps_kt[:T, P:C], k_c[:, 1, :], ident)
        kt_sb = ktpool.tile([T, C], _DT, tag="kt")
        nc.scalar.copy(out=kt_sb, in_=ps_kt[:T, :C])

        # kWT[s, c] = sum_t W[t, s] kT[t, c]
        ps_kwt = psum_sc.tile([P, 512], _DT, tag="ps_sc")
        nc.tensor.matmul(ps_kwt[:T, :C], lhsT=w_sb, rhs=kt_sb, start=True, stop=True)
        kwt_sb = ktpool.tile([T, C], _DT, tag="kwt")
        nc.scalar.copy(out=kwt_sb, in_=ps_kwt[:T, :C])

        # scoresT[d, c] = sum_s kT[s, d] kWT[s, c]  (= scores[c, d]);
        # attT = exp(scoresT), written directly as float32r matmul weights.
        ps_sc = psum_sc.tile([P, 512], _DT, tag="ps_sc")
        att_t = attpool.tile([P, CC, C], _R, tag="attT")
        for dc in range(CC):
            nc.tensor.matmul(
                ps_sc[:, ts(dc, C)], lhsT=kt_sb[:, ts(dc, P)], rhs=kwt_sb,
                start=True, stop=True,
            )
        for dc in range(CC):
            nc.scalar.activation(
                out=att_t[:, dc, :],
                in_=ps_sc[:, ts(dc, C)],
                func=mybir.ActivationFunctionType.Exp,
            )
        st["att_t"] = att_t

    def phase2(b, st):
        """Big matmul out[c, f] (+ denominator column), normalize, store."""
        x_t, att_t = st["x_t"], st["att_t"]
        for cc in range(CC):
            rinv = rpool.tile([P, 1], _DT, tag="rinv")
            o_row = outpool.tile([P, F], _DT, tag="o")
            for wi, wave in enumerate(_WAVES):
                tiles = [
                    psum.tile([P, 512], _DT, tag="ps", name=f"ps_w{wi}_{i}")
                    for i in range(len(wave))
                ]
                # zigzag dc order: adjacent waves share the stationary at
                # the boundary, so ldw-opt drops that LDWEIGHTS.
                dcs = (0, 1) if wi % 2 == 0 else (1, 0)
                for j, dc in enumerate(dcs):
                    for (f0, fsz), pt in zip(wave, tiles):
                        nc.tensor.matmul(
                            pt[:, :fsz],
                            lhsT=att_t[:, dc, ts(cc, P)],
                            rhs=x_t[:, dc, f0 : f0 + fsz],
                            start=(j == 0),
                            stop=(j == 1),
                        )
                if wi == 0:
                    # output col 3684 (offset 450 of tile 0) holds the
                    # denominator sum_d exp(scores[c, d])
                    nc.vector.reciprocal(
                        out=rinv, in_=tiles[0][:, _DEN_OFF : _DEN_OFF + 1]
                    )
                for (f0, fsz), pt in zip(wave, tiles):
                    osz = min(fsz, F - f0)  # drop the ones-columns
                    nc.scalar.mul(
                        out=o_row[:, f0 : f0 + osz], in_=pt[:, :osz], mul=rinv
                    )
            # one full-row store per c-chunk, issued from the ACT ring so
            # it can never queue behind a blocked x-load issue.
            nc.scalar.dma_start(out=out_flat[b, ts(cc, P), :], in_=o_row)

    # Three-stage software pipeline.  Per step: the big matmul of b-3
    # first (its inputs are long ready, keeps PE/ACT/DMA busy), then the
    # short scores chain of b-2 (k had a full period to finish), then
    # loads + k of b.
    states = {}
    for s in range(B_LOC + 3):
        if 0 <= s - 3:
            phase2(s - 3, states.pop(s - 3))
        if 0 <= s - 2 < B_LOC:
            phase1b(s - 2, states[s - 2])
        if s < B_LOC:
            states[s] = phase1a(s)
    ctx.close()


_CACHED_NC = None


def _build():
    global _CACHED_NC
    if _CACHED_NC is not None:
        return _CACHED_NC
    _enable_ldw_opt()
    nc = bacc.Bacc("TRN2", target_bir_lowering=False, debug=False, num_devices=NCORES)
    x_d = nc.dram_tensor("x", [B_LOC, C, N, T], _DT, kind="ExternalInput").ap()
    w_d = nc.dram_tensor("W", [T, T], _DT, kind="ExternalInput").ap()
    a_d = nc.dram_tensor("alpha", [N], _DT, kind="ExternalInput").ap()
    o_d = nc.dram_tensor("out", [B_LOC, C, N, T], _DT, kind="ExternalOutput").ap()
    with tile.TileContext(nc) as tc:
        _emit_core_kernel(tc, x_d, w_d, a_d, o_d)
    nc.compile()
    _CACHED_NC = nc
    return nc


def run(x, W, alpha, trace=False, **spmd_kwargs):
    """Run on 8 cores; returns (full output [B,C,N,T], BassKernelResults)."""
    x = np.ascontiguousarray(np.asarray(x, dtype=np.float32))
    W = np.ascontiguousarray(np.asarray(W, dtype=np.float32))
    alpha = np.ascontiguousarray(np.asarray(alpha, dtype=np.float32))
    assert x.shape == (B, C, N, T) and W.shape == (T, T) and alpha.shape == (N,)

    nc = _build()
    in_maps = [
        {"x": x[i * B_LOC : (i + 1) * B_LOC], "W": W, "alpha": alpha}
        for i in range(NCORES)
    ]
    res = run_bass_kernel_spmd(
        nc, in_maps, core_ids=list(range(NCORES)), trace=trace, **spmd_kwargs
    )
    out = np.concatenate([r["out"] for r in res.results], axis=0)
    return out, res


def kernel(x, W, alpha):
    out, _ = run(x, W, alpha)
    return out
